# revision 27
# baseline (speedup 1.0000x reference)
"""AttentionHeadCheb distributed Trainium2 kernel (8 NeuronCores).

Destination-node sharding across 8 cores; 4 source-chunk phases
(CHUNK=12500). Per (row,chunk) edge runs padded to x4; packed into 16
segs of 2560 edges per phase (4 reserved pad edges per seg; groups never
straddle segs). Score-land processes 8 blocks of W=5120 edges across the
8 partition groups. Denominators accumulate into a full-partition dacc
via scan-end gathers; msum via per-seg mask-restart scans + window end
gathers.
"""

import numpy as np
import ml_dtypes

BF16 = ml_dtypes.bfloat16

N_NODES = 50000
IN_DIM = 128
OUT_DIM = 64
NC = 8
NLOC = N_NODES // NC          # 6250
NPH = 4
CHUNK = N_NODES // NPH        # 12500
W4 = 4
SEG = 2304                    # packing unit (4 reserved pad slots at start)
W = 2 * SEG                   # block width = ap_gather group = 5120
NBLK = 8
TPH = W * NBLK                # 40960 edge slots per phase
PW = W // W4                  # 1280 partials per block
PTOT = TPH // W4              # 10240 partials per phase
PWIN = 2560                   # partials per window (2 blocks)
NWIN = 4
NLE = 6256                    # NLOC padded to x16
BIAS_PAD = -60.0
SUBA = 1152                   # score-land sub-chunk (edges)
SUBM = 2304                   # main sub-chunk (= one seg = 576 partials)


def _pack_weights(W_transform, w_left, w_right, W_residual):
    W01 = np.concatenate([W_transform[0], W_transform[1]], axis=1)  # [128,128]
    LAL = np.zeros((128, 128), np.float32)
    LAR = np.zeros((128, 16), np.float32)
    for i in range(3):
        LAL[0:64, i::16] = w_left[0][i][:, None]
        LAL[64:128, (4 + i)::16] = w_left[1][i][:, None]
        LAR[0:64, i] = w_right[0][i]
        LAR[64:128, 4 + i] = w_right[1][i]
    LALX = W01 @ LAL            # al = x @ LALX directly
    LARX = W01 @ LAR
    WRT = W_residual[0:IN_DIM]
    WRB = np.concatenate([W_residual[IN_DIM:], W_residual[IN_DIM:]], axis=0)
    # CSEL[p, o] = 1 iff out-row o (= 16g+4k) sums partitions 16g+4k..+3
    CSEL = np.zeros((128, 128), np.float32)
    for g in range(8):
        for k in range(2):
            CSEL[16 * g + 4 * k:16 * g + 4 * k + 4, 16 * g + 4 * k] = 1.0
    # DSEL2[p, o] = 1 iff p = 16g+4k and (o<64) == (k==0): denom broadcast
    DSEL2 = np.zeros((128, 128), np.float32)
    for g in range(8):
        DSEL2[16 * g + 0, 0:64] = 1.0
        DSEL2[16 * g + 4, 64:128] = 1.0
    # W2: row 0 -> out partitions 0:64, row 1 -> 64:128 (exrep broadcast)
    W2 = np.zeros((2, 128), np.float32)
    W2[0, 0:64] = 1.0
    W2[1, 64:128] = 1.0
    return (W01.astype(BF16), LALX.astype(BF16), LARX.astype(BF16),
            WRT.astype(BF16), WRB.astype(BF16), CSEL.astype(BF16),
            DSEL2.astype(BF16), W2.astype(BF16))


def _wrap16_rep(vals, nidx):
    v = vals.reshape(nidx // 16, 16).T
    return np.tile(v, (8, 1)).astype(np.int16)


def _wrap16_grouped(vals):
    g, eb = vals.shape
    out = np.empty((16 * g, eb // 16), np.int16)
    for gg in range(g):
        out[16 * gg:16 * gg + 16] = vals[gg].reshape(eb // 16, 16).T
    return out


def _prep_core(m, r, c, atten_vals, support_vals):
    sel = np.where((r >= m * NLOC) & (r < (m + 1) * NLOC))[0]
    rl = (r[sel] - m * NLOC).astype(np.int64)
    cg = c[sel].astype(np.int64)
    ch = (cg // CHUNK).astype(np.int64)
    order = np.lexsort((ch, rl))
    sel, rl, cg, ch = sel[order], rl[order], cg[order], ch[order]
    cl = (cg % CHUNK).astype(np.int64)

    phases = []
    for pc in range(NPH):
        pm = ch == pc
        prl, pcl, psel = rl[pm], cl[pm], sel[pm]
        ne = prl.size
        gstart = np.flatnonzero(np.r_[True, prl[1:] != prl[:-1]]) if ne else \
            np.zeros(0, np.int64)
        gcnt = np.diff(np.r_[gstart, ne]) if ne else np.zeros(0, np.int64)
        grow = prl[gstart] if ne else np.zeros(0, np.int64)
        gpad = ((gcnt + 3) // 4) * 4
        ng = grow.size
        gpos = np.empty(ng, np.int64)       # global slot of group start
        seg_i, off = 0, 4
        NSEG = TPH // SEG
        for i in range(ng):
            if off + gpad[i] > SEG:
                seg_i += 1
                off = 4
            assert seg_i < NSEG, f"core {m} phase {pc}: seg overflow"
            gpos[i] = seg_i * SEG + off
            off += gpad[i]
        within = np.arange(ne) - np.repeat(gstart, gcnt)
        slot = np.repeat(gpos, gcnt) + within
        cols = np.zeros(TPH, np.int64)
        rows = np.zeros(TPH, np.int64)
        vrow = np.zeros((8, TPH), np.float32)
        vrow[3] = BIAS_PAD
        vrow[7] = BIAS_PAD
        cols[slot] = pcl
        rows[slot] = prl
        e0 = psel
        vrow[0][slot] = atten_vals[0][e0]
        vrow[1][slot] = atten_vals[1][e0]
        vrow[2][slot] = support_vals[0][e0]
        vrow[3][slot] = 0.0
        vrow[4][slot] = atten_vals[0][e0]
        vrow[5][slot] = atten_vals[1][e0]
        vrow[6][slot] = support_vals[1][e0]
        vrow[7][slot] = 0.0
        # edge segment ids (pads negative per seg)
        esid = np.zeros(TPH, np.int64)
        for si in range(NSEG):
            esid[si * SEG:(si + 1) * SEG] = -(si + 1)
        gp_hi = gpos + gpad
        for i in range(ng):
            esid[gpos[i]:gp_hi[i]] = i
        emask = np.ones(TPH, np.float32)
        emask[0] = 0.0
        emask[1:][esid[1:] != esid[:-1]] = 0.0
        emask[0::SEG] = 0.0
        psid = esid[0::W4]
        pmask = np.ones(TPH // W4, np.float32)
        pmask[0] = 0.0
        pmask[1:][psid[1:] != psid[:-1]] = 0.0
        pmask[0::SEG // W4] = 0.0
        # msgs ends: global partial position within the phase
        pend = gp_hi // W4 - 1
        endq = np.zeros(NLE, np.int64)
        for i in range(ng):
            endq[grow[i]] = pend[i]
        # denom ends: block-local end edge (exs table [*, W])
        dend = np.zeros((8, NLE), np.int64)      # [group, row]
        gblk = gpos // W
        eloc = (gp_hi - 1) % W
        for i in range(ng):
            dend[gblk[i], grow[i]] = eloc[i]
        colw = np.empty((128, TPH // 16), np.int16)
        for t in range(NBLK):
            colw[:, t * (W // 16):(t + 1) * (W // 16)] = _wrap16_rep(
                cols[t * W:(t + 1) * W], W)
        rloc = _wrap16_grouped(rows.reshape(8, W))
        cloc = _wrap16_grouped(cols.reshape(8, W))
        endqw = _wrap16_rep(endq, NLE)
        dendw = _wrap16_grouped(dend)
        vst = np.zeros((128, W), BF16)
        for g in range(8):
            for i in range(8):
                vst[16 * g + i] = vrow[i][g * W:(g + 1) * W].astype(BF16)
        pmrep = np.broadcast_to(pmask.astype(BF16)[None, :],
                                (128, PTOT)).copy()
        emrep = np.repeat(emask.reshape(8, W).astype(BF16), 16, axis=0)
        ph = dict(colw=colw, rloc=rloc, cloc=cloc,
                  pmrep=pmrep, emrep=emrep, vst=vst,
                  endqw=endqw, dendw=dendw)
        # emulation-only helpers
        ph["_emask8"] = emask.reshape(8, W).astype(np.float32)
        ph["_pmask8"] = pmask.reshape(8, PW).astype(np.float32)
        phases.append(ph)
    return phases


def host_prep(x, support_vals, atten_vals, W_transform, w_left, w_right,
              W_residual, edge_rows, edge_cols):
    W01, LALX, LARX, WRT, WRB, CSEL, DSEL2, W2 = _pack_weights(
        W_transform, w_left, w_right, W_residual)
    ONESROW = np.ones((1, NLOC), np.float32)
    in_maps = []
    for m in range(NC):
        ph = _prep_core(m, edge_rows, edge_cols, atten_vals, support_vals)
        xT = np.ascontiguousarray(x[m * NLOC:(m + 1) * NLOC].T).astype(BF16)
        im = dict(xT=xT, W01=W01, LALX=LALX, LARX=LARX, WRT=WRT, WRB=WRB,
                  CSEL=CSEL, DSEL2=DSEL2, W2=W2, ONESROW=ONESROW)
        for pc in range(NPH):
            for k, v in ph[pc].items():
                if k.startswith("_"):
                    im[f"{k}{pc}"] = v      # emulation-only, not a dram input
                else:
                    im[f"{k}{pc}"] = np.ascontiguousarray(v)
        in_maps.append(im)
    return in_maps


# ======================================================================
# Numpy emulation
# ======================================================================

def emulate(in_maps, x, W_transform, w_left, w_right, W_residual):
    xb = x.astype(BF16).astype(np.float32)
    wx_all = np.concatenate(
        [xb @ W_transform[k].astype(BF16).astype(np.float32)
         for k in range(2)], axis=1)
    wxT = wx_all.T
    ar_all = np.zeros((16, N_NODES), np.float32)
    al_all = np.zeros((16, N_NODES), np.float32)
    for k in range(2):
        ar_all[4 * k:4 * k + 3] = (wx_all[:, 64 * k:64 * k + 64] @
                                   w_right[k].T).T
        al_all[4 * k:4 * k + 3] = (wx_all[:, 64 * k:64 * k + 64] @
                                   w_left[k].T).T
    al_all[3] = 1.0
    al_all[7] = 1.0

    def segscan(parts, mrow):
        cs = np.cumsum(parts, axis=-1)
        starts = np.flatnonzero(mrow == 0.0)
        seg = np.cumsum(mrow == 0.0) - 1
        offs = np.take(cs[..., starts] - parts[..., starts], seg, axis=-1)
        return cs - offs

    outs = []
    for m in range(NC):
        im = in_maps[m]
        al_loc = al_all[:, m * NLOC:(m + 1) * NLOC]
        msum = np.zeros((128, NLOC), np.float64)
        dsum = np.zeros((2, NLOC), np.float64)
        for pc in range(NPH):
            rloc = im[f"rloc{pc}"].astype(np.int64)
            cloc = im[f"cloc{pc}"].astype(np.int64)
            alo = np.zeros((128, W), np.float32)
            aro = np.zeros((128, W), np.float32)
            for g in range(8):
                idx = rloc[16 * g:16 * g + 16].T.reshape(-1)
                alo[16 * g:16 * g + 16] = al_loc[:, idx]
                idxc = cloc[16 * g:16 * g + 16].T.reshape(-1)
                aro[16 * g:16 * g + 16] = ar_all[:, pc * CHUNK + idxc]
            alo = alo.astype(BF16).astype(np.float32)
            aro = aro.astype(BF16).astype(np.float32)
            p8 = (alo + aro) * im[f"vst{pc}"].astype(np.float32)
            s = p8[0::4] + p8[1::4] + p8[2::4] + p8[3::4]
            ex8 = np.exp(s)                       # [32, W]
            emask = im[f"_emask8{pc}"]
            exs = np.zeros((32, W), np.float32)
            for g in range(8):
                for k in (0, 1):
                    for h in (0, 1):
                        sl = slice(h * SEG, (h + 1) * SEG)
                        exs[4 * g + k, sl] = segscan(ex8[4 * g + k, sl],
                                                     emask[g, sl])
            dendw = im[f"dendw{pc}"].astype(np.int64)
            for g in range(8):
                idx = dendw[16 * g:16 * g + 16].T.reshape(-1)
                dsum[0] += exs[4 * g + 0, idx][:NLOC]
                dsum[1] += exs[4 * g + 1, idx][:NLOC]
            colw = im[f"colw{pc}"].astype(np.int64)
            cols = np.empty(TPH, np.int64)
            for t in range(NBLK):
                blkw = colw[0:16, t * (W // 16):(t + 1) * (W // 16)]
                cols[t * W:(t + 1) * W] = blkw.T.reshape(-1)
            pmask = im[f"_pmask8{pc}"]
            scanq = np.zeros((128, PTOT), np.float32)
            for t in range(8):
                idx = pc * CHUNK + cols[t * W:(t + 1) * W]
                g = wxT[:, idx]
                g = g * np.where((np.arange(128) < 64)[:, None],
                                 ex8[4 * t + 0], ex8[4 * t + 1])
                g = g.astype(BF16).astype(np.float32)
                part = g.reshape(128, PW, W4).sum(2)
                scanq[:, t * PW:(t + 1) * PW] = segscan(part, pmask[t])
            endw = im[f"endqw{pc}"].astype(np.int64)
            eidx = endw[0:16].T.reshape(-1)
            msum += scanq[:, eidx[:NLOC]]
        dsum += 1e-30
        out01 = msum.copy()
        out01[0:64] /= dsum[0]
        out01[64:128] /= dsum[1]
        xs = xb[m * NLOC:(m + 1) * NLOC]
        pre = (xs @ W_residual[:IN_DIM] +
               (out01[0:64] + out01[64:128]).T @ W_residual[IN_DIM:])
        out = np.where(pre > 0, pre, np.exp(np.minimum(pre, 0)) - 1)
        outs.append(out.astype(np.float32))
    return np.concatenate(outs, axis=0)


# ======================================================================
# Bass kernel builder
# ======================================================================

def build_bass():
    import sys
    if '/opt/trn_rl_repo' not in sys.path:
        sys.path.insert(0, '/opt/trn_rl_repo')
    from concourse import bass, bacc, tile, mybir

    dt = mybir.dt
    AL = mybir.AluOpType
    AF = mybir.ActivationFunctionType

    nc = bacc.Bacc(None, target_bir_lowering=False)

    def din(name, shape, d):
        return nc.dram_tensor(name, list(shape), d, kind="ExternalInput")

    xT_d = din("xT", (128, NLOC), dt.bfloat16)
    W01_d = din("W01", (128, 128), dt.bfloat16)
    LALX_d = din("LALX", (128, 128), dt.bfloat16)
    LARX_d = din("LARX", (128, 16), dt.bfloat16)
    WRT_d = din("WRT", (128, 64), dt.bfloat16)
    WRB_d = din("WRB", (128, 64), dt.bfloat16)
    CSEL_d = din("CSEL", (128, 128), dt.bfloat16)
    DSEL2_d = din("DSEL2", (128, 128), dt.bfloat16)
    W2_d = din("W2", (2, 128), dt.bfloat16)
    ONESROW_d = din("ONESROW", (1, NLOC), dt.float32)
    ph_d = []
    for pc in range(NPH):
        dd = dict(
            colw=din(f"colw{pc}", (128, TPH // 16), dt.int16),
            rloc=din(f"rloc{pc}", (128, W // 16), dt.int16),
            cloc=din(f"cloc{pc}", (128, W // 16), dt.int16),
            vst=din(f"vst{pc}", (128, W), dt.bfloat16),
            pmrep=din(f"pmrep{pc}", (128, PTOT), dt.bfloat16),
            emrep=din(f"emrep{pc}", (128, W), dt.bfloat16),
        )
        dd["endqw"] = din(f"endqw{pc}", (128, NLE // 16), dt.int16)
        dd["dendw"] = din(f"dendw{pc}", (128, NLE // 16), dt.int16)
        ph_d.append(dd)
    out_d = nc.dram_tensor("out", [64, NLOC], dt.float32,
                           kind="ExternalOutput")
    aginw = nc.dram_tensor("aginw", [NLOC, 128], dt.bfloat16)
    agoutw = nc.dram_tensor("agoutw", [N_NODES, 128], dt.bfloat16,
                            addr_space="Shared")
    aginr = nc.dram_tensor("aginr", [16, NLOC], dt.float32)
    agoutr = nc.dram_tensor("agoutr", [16 * NC, NLOC], dt.float32,
                            addr_space="Shared")

    NT512 = (NLOC + 511) // 512
    NJ = (NLE + 511) // 512

    with tile.TileContext(nc) as tc:
      with nc.allow_low_precision(reason="bf16 accums validated in emulation"):
        with (
            tc.tile_pool(name="big", bufs=1) as big,
            tc.tile_pool(name="alr", bufs=1) as alr,
            tc.tile_pool(name="res", bufs=1) as res,
            tc.tile_pool(name="mid", bufs=1) as mid,
            tc.tile_pool(name="work", bufs=2) as work,
            tc.tile_pool(name="one", bufs=1) as one,
            tc.tile_pool(name="psum", bufs=2, space="PSUM") as psum,
        ):
            # ---------- stage 1: wx, al, ar, collective ----------
            altab = alr.tile([128, NLOC], dt.float32, tag="altab")
            W01 = mid.tile([128, 128], dt.bfloat16, tag="w128")
            LALXt = mid.tile([128, 128], dt.bfloat16, tag="w128b")
            LARXt = mid.tile([128, 16], dt.bfloat16, tag="w16")
            nc.sync.dma_start(W01[:], W01_d[:])
            nc.sync.dma_start(LALXt[:], LALX_d[:])
            nc.sync.dma_start(LARXt[:], LARX_d[:])
            for j in range(NT512):
                a, b = j * 512, min(NLOC, (j + 1) * 512)
                xch = one.tile([128, 512], dt.bfloat16, tag="vmask")
                nc.sync.dma_start(xch[:, :b - a], xT_d[:, a:b])
                pw = psum.tile([128, 512], dt.float32, tag="pw")
                nc.tensor.matmul(pw[:, :b - a], W01[:], xch[:, :b - a],
                                 start=True, stop=True)
                wxb = one.tile([128, 512], dt.bfloat16, tag="p8x")
                nc.vector.tensor_copy(wxb[:, :b - a], pw[:, :b - a])
                for j4 in range(4):
                    na = a + j4 * 128
                    nb = min(b, na + 128)
                    if nb <= na:
                        continue
                    ntile = work.tile([128, 128], dt.bfloat16, tag="ntile")
                    nc.sync.dma_start(ntile[:],
                                      wxb[:, j4 * 128:(j4 + 1) * 128],
                                      transpose=True)
                    nc.scalar.dma_start(aginw[na:nb, :], ntile[:nb - na, :])
                pa = psum.tile([128, 512], dt.float32, tag="pw")
                nc.tensor.matmul(pa[:, :b - a], LALXt[:], xch[:, :b - a],
                                 start=True, stop=True)
                nc.scalar.activation(altab[:, a:b], pa[:, :b - a], AF.Copy)
                pr = psum.tile([16, 512], dt.float32, tag="pw")
                nc.tensor.matmul(pr[:, :b - a], LARXt[:], xch[:, :b - a],
                                 start=True, stop=True)
                ar16s = one.tile([16, 512], dt.float32, tag="ppseg")
                nc.scalar.activation(ar16s[:, :b - a], pr[:, :b - a], AF.Copy)
                nc.sync.dma_start(aginr[:, a:b], ar16s[:, :b - a])
            for g8 in range(8):
                nc.sync.dma_start(altab[16 * g8 + 3:16 * g8 + 4, :],
                                  ONESROW_d[:])
                nc.sync.dma_start(altab[16 * g8 + 7:16 * g8 + 8, :],
                                  ONESROW_d[:])
            nc.gpsimd.collective_compute(
                "AllGather", AL.bypass,
                replica_groups=[list(range(NC))],
                ins=[aginw.ap().opt()],
                outs=[agoutw.ap().opt()],
            )
            nc.gpsimd.collective_compute(
                "AllGather", AL.bypass,
                replica_groups=[list(range(NC))],
                ins=[aginr.ap().opt()],
                outs=[agoutr.ap().opt()],
            )

            msum = res.tile([128, NLOC], dt.bfloat16, tag="msum")
            dacc = res.tile([128, NLE], dt.bfloat16, tag="dacc")
            nc.vector.memset(msum[:], 0.0)
            nc.vector.memset(dacc[:], 0.0)
            CSELt = mid.tile([128, 128], dt.bfloat16, tag="csel")
            nc.sync.dma_start(CSELt[:], CSEL_d[:])
            W2t = mid.tile([2, 128], dt.bfloat16, tag="w2")
            nc.sync.dma_start(W2t[:], W2_d[:])

            for pc in range(NPH):
                pd = ph_d[pc]
                rloc = mid.tile([128, W // 16], dt.int16, tag="rloc")
                cloc = mid.tile([128, W // 16], dt.int16, tag="cloc")
                nc.sync.dma_start(rloc[:], pd["rloc"][:])
                nc.sync.dma_start(cloc[:], pd["cloc"][:])
                # --- B: score-land on the (time-shared) big slot: artab ---
                artab = big.tile([128, CHUNK], dt.float32, tag="big")
                for jj in range(2):
                    rk = 2 * pc + jj
                    for g in range(8):
                        nc.scalar.dma_start(
                            artab[16 * g:16 * g + 16,
                                  jj * NLOC:(jj + 1) * NLOC],
                            agoutr[rk * 16:rk * 16 + 16, :])
                ex8 = res.tile([128, W], dt.bfloat16, tag="ex8")
                alb16 = one.tile([128, W], dt.bfloat16, tag="alo")
                for s in range(W // SUBA):
                    a, b = s * SUBA, (s + 1) * SUBA
                    sw = SUBA // 16
                    albuf = one.tile([128, 2560], dt.float32, tag="gath")
                    nc.gpsimd.ap_gather(albuf[:, 0:SUBA], altab[:],
                                        rloc[:, s * sw:(s + 1) * sw],
                                        channels=128, num_elems=NLOC, d=1,
                                        num_idxs=SUBA)
                    nc.vector.tensor_copy(alb16[:, a:b], albuf[:, 0:SUBA])
                for s in range(W // SUBA):
                    a, b = s * SUBA, (s + 1) * SUBA
                    sw = SUBA // 16
                    aro = one.tile([128, 2560], dt.float32, tag="gath")
                    nc.gpsimd.ap_gather(aro[:, 0:SUBA], artab[:],
                                        cloc[:, s * sw:(s + 1) * sw],
                                        channels=128, num_elems=CHUNK, d=1,
                                        num_idxs=SUBA)
                    p8 = one.tile([128, SUBA], dt.bfloat16, tag="p8x")
                    nc.vector.tensor_tensor(p8[:], aro[:, 0:SUBA],
                                            alb16[:, a:b], AL.add)
                    vsts = one.tile([128, 2560], dt.bfloat16, tag="vmask")
                    nc.sync.dma_start(vsts[:, 0:SUBA], pd["vst"][:, a:b])
                    nc.vector.tensor_tensor(p8[:], p8[:],
                                            vsts[:, 0:SUBA], AL.mult)
                    for va, vb in ((0, 512), (512, 1024), (1024, 1152)):
                        sxp = psum.tile([128, 512], dt.float32, tag="pw")
                        nc.tensor.matmul(sxp[:, :vb - va], CSELt[:],
                                         p8[:, va:vb],
                                         start=True, stop=True)
                        nc.scalar.activation(ex8[:, a + va:a + vb],
                                             sxp[:, :vb - va], AF.Exp)
                # wxtab load can overlap the rest of B (WAR on big slot)
                exs = res.tile([128, W], dt.float32, tag="exsf")
                for h in (0, 1):
                    emaskh = one.tile([128, 2560], dt.bfloat16, tag="vmask")
                    nc.sync.dma_start(emaskh[:, 0:SEG],
                                      pd["emrep"][:, h * SEG:(h + 1) * SEG])
                    hs = slice(h * SEG, (h + 1) * SEG)
                    nc.vector.tensor_tensor_scan(
                        exs[:, hs], emaskh[:, 0:SEG], ex8[:, hs], 0.0,
                        op0=AL.mult, op1=AL.add)
                dendw = mid.tile([128, NLE // 16], dt.int16, tag="endidx")
                nc.sync.dma_start(dendw[:], pd["dendw"][:])
                for ea, en in ((0, 3136), (3136, 3120)):
                    endout = one.tile([128, 3136], dt.float32, tag="alo")
                    nc.gpsimd.ap_gather(
                        endout[:, 0:en], exs[:],
                        dendw[:, ea // 16:(ea + en) // 16],
                        channels=128, num_elems=W, d=1, num_idxs=en)
                    nc.vector.tensor_tensor(dacc[:, ea:ea + en],
                                            dacc[:, ea:ea + en],
                                            endout[:, 0:en], AL.add)
                # --- C: main gather (transposed dma_gather) + msgs ---

                scanq = res.tile([128, PTOT], dt.float32, tag="sh16")
                for q in range(NWIN):
                    for tt in range(2):
                        t = 2 * q + tt
                        for sg in range(2):      # segs (= SUBM) within block
                            e0 = t * W + sg * SEG
                            w0 = e0 // 16
                            gt = one.tile([128, 9, 1, 256], dt.bfloat16,
                                          tag="gtt")
                            colws = work.tile([128, SUBM // 16], dt.int16,
                                              tag="colws")
                            nc.sync.dma_start(colws[:],
                                              pd["colw"][:, w0:w0 + SUBM // 16])
                            for dg in range(9):
                                nc.gpsimd.dma_gather(
                                    gt[:, dg, :, :],
                                    agoutw[pc * CHUNK:(pc + 1) * CHUNK, :],
                                    colws[:, dg * 16:(dg + 1) * 16],
                                    num_idxs=256, num_idxs_reg=256,
                                    elem_size=128, transpose=True)
                            gt = gt[:].rearrange("p a b c -> p (a b c)")
                            c0 = sg * SEG
                            ppseg = one.tile([128, SUBM // 4], dt.float32,
                                             tag="ppseg")
                            for v2 in range(2):
                                va = v2 * 1152
                                exfm = one.tile([2, 1152], dt.bfloat16,
                                                tag="p8x")
                                nc.sync.dma_start(
                                    exfm[0:1, :],
                                    ex8[16 * t:16 * t + 1,
                                        c0 + va:c0 + va + 1152])
                                nc.sync.dma_start(
                                    exfm[1:2, :],
                                    ex8[16 * t + 4:16 * t + 5,
                                        c0 + va:c0 + va + 1152])
                                exrep = psum.tile([128, 1152], dt.float32,
                                                  tag="exrep")
                                for v3 in range(3):
                                    sa = v3 * 512
                                    sb = min(1152, sa + 512)
                                    nc.tensor.matmul(
                                        exrep[:, sa:sb], W2t[:],
                                        exfm[:, sa:sb],
                                        start=True, stop=True)
                                gb = one.tile([128, 288, 4],
                                              dt.bfloat16, tag="gb")
                                g2o = gb[:].rearrange("p a b -> p (a b)")
                                nc.vector.tensor_tensor(
                                    g2o[:, :], gt[:, va:va + 1152],
                                    exrep[:], AL.mult)
                                nc.vector.tensor_reduce(
                                    ppseg[:, v2 * 288:(v2 + 1) * 288],
                                    gb[:], axis=mybir.AxisListType.X,
                                    op=AL.add)
                            pb = t * PW + sg * (PW // 2)
                            pglob = t * PW + sg * (PW // 2)
                            mkrs = one.tile([128, 2560], dt.bfloat16,
                                            tag="vmask")
                            nc.sync.dma_start(
                                mkrs[:, 0:PW // 2],
                                pd["pmrep"][:, pglob:pglob + PW // 2])
                            nc.vector.tensor_tensor_scan(
                                scanq[:, pb:pb + PW // 2],
                                mkrs[:, 0:PW // 2], ppseg[:],
                                0.0, op0=AL.mult, op1=AL.add)
                endw = mid.tile([128, NLE // 16], dt.int16, tag="endidx")
                nc.sync.dma_start(endw[:], pd["endqw"][:])
                for ea, en in ((0, 3136), (3136, 3120)):
                    bb = min(NLOC, ea + en)
                    endout2 = one.tile([128, 3136], dt.float32, tag="alo")
                    nc.gpsimd.ap_gather(
                        endout2[:, 0:en], scanq[:],
                        endw[:, ea // 16:(ea + en) // 16],
                        channels=128, num_elems=PTOT, d=1, num_idxs=en)
                    nc.vector.tensor_tensor(
                        msum[:, ea:bb], msum[:, ea:bb],
                        endout2[:, 0:bb - ea], AL.add)

            # ---------- stage 4: divide + residual + elu ----------
            DSEL2t = mid.tile([128, 128], dt.bfloat16, tag="dsel2")
            nc.sync.dma_start(DSEL2t[:], DSEL2_d[:])
            WRTt = mid.tile([128, 64], dt.bfloat16, tag="w128")
            WRBt = mid.tile([128, 64], dt.bfloat16, tag="w128b")
            nc.sync.dma_start(WRTt[:], WRT_d[:])
            nc.sync.dma_start(WRBt[:], WRB_d[:])
            for j in range(NT512):
                a, b = j * 512, min(NLOC, (j + 1) * 512)
                xch = one.tile([128, 512], dt.bfloat16, tag="vmask")
                nc.sync.dma_start(xch[:, :b - a], xT_d[:, a:b])
                drp = psum.tile([128, 512], dt.float32, tag="pw")
                nc.tensor.matmul(drp[:, :b - a], DSEL2t[:], dacc[:, a:b],
                                 start=True, stop=True)
                rec = one.tile([128, PW // 2], dt.float32, tag="ppseg")
                nc.vector.tensor_scalar(rec[:, :b - a], drp[:, :b - a],
                                        1e-8, None, AL.add)
                nc.vector.reciprocal(rec[:, :b - a], rec[:, :b - a])
                msb = one.tile([128, 512], dt.bfloat16, tag="gb")
                nc.vector.tensor_tensor(msb[:, :b - a], msum[:, a:b],
                                        rec[:, :b - a], AL.mult)
                prr = psum.tile([64, 512], dt.float32, tag="pw")
                nc.tensor.matmul(prr[:, :b - a], WRTt[:], xch[:, :b - a],
                                 start=True, stop=False)
                nc.tensor.matmul(prr[:, :b - a], WRBt[:], msb[:, :b - a],
                                 start=False, stop=True)
                et = one.tile([128, PW // 2], dt.float32, tag="ppseg")
                nc.scalar.activation(et[0:64, :b - a], prr[:, :b - a], AF.Exp)
                nc.vector.tensor_scalar(et[0:64, :b - a], et[0:64, :b - a],
                                        -1.0, 0.0, AL.add, AL.min)
                nc.vector.tensor_scalar(prr[:, :b - a], prr[:, :b - a],
                                        0.0, None, AL.max)
                osb = one.tile([64, 512], dt.float32, tag="gb")
                nc.vector.tensor_tensor(osb[:, :b - a], et[0:64, :b - a],
                                        prr[:, :b - a], AL.add)
                nc.sync.dma_start(out_d[:, a:b], osb[:, :b - a])

    nc.compile()
    return nc


_CACHED = {}


def kernel(**inputs):
    import sys
    if '/opt/trn_rl_repo' not in sys.path:
        sys.path.insert(0, '/opt/trn_rl_repo')
    from concourse import bass_utils

    np_inputs = {k: np.asarray(v) for k, v in inputs.items()}
    in_maps = host_prep(**np_inputs)
    in_maps = [{k: v for k, v in im.items() if not k.startswith("_")}
               for im in in_maps]
    if 'nc' not in _CACHED:
        _CACHED['nc'] = build_bass()
    nc = _CACHED['nc']
    res = bass_utils.run_bass_kernel_spmd(nc, in_maps,
                                          core_ids=list(range(NC)))
    outs = [res.results[m]["out"] for m in range(NC)]
    return np.concatenate([o.T for o in outs], axis=0).astype(np.float32)


# revision 28
# speedup vs baseline: 1.1655x; 1.1655x over previous
"""AttentionHeadCheb distributed Trainium2 kernel (8 NeuronCores).

Destination-node sharding across 8 cores; 4 source-chunk phases
(CHUNK=12500). Per (row,chunk) edge runs padded to x4; packed into 16
segs of 2560 edges per phase (4 reserved pad edges per seg; groups never
straddle segs). Score-land processes 8 blocks of W=5120 edges across the
8 partition groups. Denominators accumulate into a full-partition dacc
via scan-end gathers; msum via per-seg mask-restart scans + window end
gathers.
"""

import numpy as np
import ml_dtypes

BF16 = ml_dtypes.bfloat16

N_NODES = 50000
IN_DIM = 128
OUT_DIM = 64
NC = 8
NLOC = N_NODES // NC          # 6250
NPH = 4
CHUNK = N_NODES // NPH        # 12500
W4 = 4
SEG = 2304                    # packing unit (4 reserved pad slots at start)
W = 2 * SEG                   # block width = ap_gather group = 5120
NBLK = 8
TPH = W * NBLK                # 40960 edge slots per phase
PW = W // W4                  # 1280 partials per block
PTOT = TPH // W4              # 10240 partials per phase
PWIN = 2560                   # partials per window (2 blocks)
NWIN = 4
NLE = 6256                    # NLOC padded to x16
BIAS_PAD = -60.0
SUBA = 1152                   # score-land sub-chunk (edges)
SUBM = 2304                   # main sub-chunk (= one seg = 576 partials)


def _pack_weights(W_transform, w_left, w_right, W_residual):
    W01 = np.concatenate([W_transform[0], W_transform[1]], axis=1)  # [128,128]
    LAL = np.zeros((128, 128), np.float32)
    LAR = np.zeros((128, 16), np.float32)
    for i in range(3):
        LAL[0:64, i::16] = w_left[0][i][:, None]
        LAL[64:128, (4 + i)::16] = w_left[1][i][:, None]
        LAR[0:64, i] = w_right[0][i]
        LAR[64:128, 4 + i] = w_right[1][i]
    LALX = W01 @ LAL            # al = x @ LALX directly
    LARX = W01 @ LAR
    WRT = W_residual[0:IN_DIM]
    WRB = np.concatenate([W_residual[IN_DIM:], W_residual[IN_DIM:]], axis=0)
    # CSEL[p, o] = 1 iff out-row o (= 16g+4k) sums partitions 16g+4k..+3
    CSEL = np.zeros((128, 128), np.float32)
    for g in range(8):
        for k in range(2):
            CSEL[16 * g + 4 * k:16 * g + 4 * k + 4, 16 * g + 4 * k] = 1.0
    # DSEL2[p, o] = 1 iff p = 16g+4k and (o<64) == (k==0): denom broadcast
    DSEL2 = np.zeros((128, 128), np.float32)
    for g in range(8):
        DSEL2[16 * g + 0, 0:64] = 1.0
        DSEL2[16 * g + 4, 64:128] = 1.0
    # W2: row 0 -> out partitions 0:64, row 1 -> 64:128 (exrep broadcast)
    W2 = np.zeros((2, 128), np.float32)
    W2[0, 0:64] = 1.0
    W2[1, 64:128] = 1.0
    return (W01.astype(BF16), LALX.astype(BF16), LARX.astype(BF16),
            WRT.astype(BF16), WRB.astype(BF16), CSEL.astype(BF16),
            DSEL2.astype(BF16), W2.astype(BF16))


def _wrap16_rep(vals, nidx):
    v = vals.reshape(nidx // 16, 16).T
    return np.tile(v, (8, 1)).astype(np.int16)


def _wrap16_grouped(vals):
    g, eb = vals.shape
    out = np.empty((16 * g, eb // 16), np.int16)
    for gg in range(g):
        out[16 * gg:16 * gg + 16] = vals[gg].reshape(eb // 16, 16).T
    return out


def _prep_core(m, r, c, atten_vals, support_vals):
    sel = np.where((r >= m * NLOC) & (r < (m + 1) * NLOC))[0]
    rl = (r[sel] - m * NLOC).astype(np.int64)
    cg = c[sel].astype(np.int64)
    ch = (cg // CHUNK).astype(np.int64)
    order = np.lexsort((ch, rl))
    sel, rl, cg, ch = sel[order], rl[order], cg[order], ch[order]
    cl = (cg % CHUNK).astype(np.int64)

    phases = []
    for pc in range(NPH):
        pm = ch == pc
        prl, pcl, psel = rl[pm], cl[pm], sel[pm]
        ne = prl.size
        gstart = np.flatnonzero(np.r_[True, prl[1:] != prl[:-1]]) if ne else \
            np.zeros(0, np.int64)
        gcnt = np.diff(np.r_[gstart, ne]) if ne else np.zeros(0, np.int64)
        grow = prl[gstart] if ne else np.zeros(0, np.int64)
        gpad = ((gcnt + 3) // 4) * 4
        ng = grow.size
        gpos = np.empty(ng, np.int64)       # global slot of group start
        seg_i, off = 0, 4
        NSEG = TPH // SEG
        for i in range(ng):
            if off + gpad[i] > SEG:
                seg_i += 1
                off = 4
            assert seg_i < NSEG, f"core {m} phase {pc}: seg overflow"
            gpos[i] = seg_i * SEG + off
            off += gpad[i]
        within = np.arange(ne) - np.repeat(gstart, gcnt)
        slot = np.repeat(gpos, gcnt) + within
        cols = np.zeros(TPH, np.int64)
        rows = np.zeros(TPH, np.int64)
        vrow = np.zeros((8, TPH), np.float32)
        vrow[3] = BIAS_PAD
        vrow[7] = BIAS_PAD
        cols[slot] = pcl
        rows[slot] = prl
        e0 = psel
        vrow[0][slot] = atten_vals[0][e0]
        vrow[1][slot] = atten_vals[1][e0]
        vrow[2][slot] = support_vals[0][e0]
        vrow[3][slot] = 0.0
        vrow[4][slot] = atten_vals[0][e0]
        vrow[5][slot] = atten_vals[1][e0]
        vrow[6][slot] = support_vals[1][e0]
        vrow[7][slot] = 0.0
        # edge segment ids (pads negative per seg)
        esid = np.zeros(TPH, np.int64)
        for si in range(NSEG):
            esid[si * SEG:(si + 1) * SEG] = -(si + 1)
        gp_hi = gpos + gpad
        for i in range(ng):
            esid[gpos[i]:gp_hi[i]] = i
        emask = np.ones(TPH, np.float32)
        emask[0] = 0.0
        emask[1:][esid[1:] != esid[:-1]] = 0.0
        emask[0::SEG] = 0.0
        psid = esid[0::W4]
        pmask = np.ones(TPH // W4, np.float32)
        pmask[0] = 0.0
        pmask[1:][psid[1:] != psid[:-1]] = 0.0
        pmask[0::SEG // W4] = 0.0
        # msgs ends: global partial position within the phase
        pend = gp_hi // W4 - 1
        endq = np.zeros(NLE, np.int64)
        for i in range(ng):
            endq[grow[i]] = pend[i]
        # denom ends: block-local end edge (exs table [*, W])
        dend = np.zeros((8, NLE), np.int64)      # [group, row]
        gblk = gpos // W
        eloc = (gp_hi - 1) % W
        for i in range(ng):
            dend[gblk[i], grow[i]] = eloc[i]
        colw = np.empty((128, TPH // 16), np.int16)
        for t in range(NBLK):
            colw[:, t * (W // 16):(t + 1) * (W // 16)] = _wrap16_rep(
                cols[t * W:(t + 1) * W], W)
        rloc = _wrap16_grouped(rows.reshape(8, W))
        cloc = _wrap16_grouped(cols.reshape(8, W))
        endqw = _wrap16_rep(endq, NLE)
        dendw = _wrap16_grouped(dend)
        vst = np.zeros((128, W), BF16)
        for g in range(8):
            for i in range(8):
                vst[16 * g + i] = vrow[i][g * W:(g + 1) * W].astype(BF16)
        pmrep = np.broadcast_to(pmask.astype(BF16)[None, :],
                                (128, PTOT)).copy()
        emrep = np.repeat(emask.reshape(8, W).astype(BF16), 16, axis=0)
        ph = dict(colw=colw, rloc=rloc, cloc=cloc,
                  pmrep=pmrep, emrep=emrep, vst=vst,
                  endqw=endqw, dendw=dendw)
        # emulation-only helpers
        ph["_emask8"] = emask.reshape(8, W).astype(np.float32)
        ph["_pmask8"] = pmask.reshape(8, PW).astype(np.float32)
        phases.append(ph)
    return phases


def host_prep(x, support_vals, atten_vals, W_transform, w_left, w_right,
              W_residual, edge_rows, edge_cols):
    W01, LALX, LARX, WRT, WRB, CSEL, DSEL2, W2 = _pack_weights(
        W_transform, w_left, w_right, W_residual)
    ONESROW = np.ones((1, NLOC), np.float32)
    in_maps = []
    for m in range(NC):
        ph = _prep_core(m, edge_rows, edge_cols, atten_vals, support_vals)
        xT = np.ascontiguousarray(x[m * NLOC:(m + 1) * NLOC].T).astype(BF16)
        im = dict(xT=xT, W01=W01, LALX=LALX, LARX=LARX, WRT=WRT, WRB=WRB,
                  CSEL=CSEL, DSEL2=DSEL2, W2=W2, ONESROW=ONESROW)
        for pc in range(NPH):
            for k, v in ph[pc].items():
                if k.startswith("_"):
                    im[f"{k}{pc}"] = v      # emulation-only, not a dram input
                else:
                    im[f"{k}{pc}"] = np.ascontiguousarray(v)
        in_maps.append(im)
    return in_maps


# ======================================================================
# Numpy emulation
# ======================================================================

def emulate(in_maps, x, W_transform, w_left, w_right, W_residual):
    xb = x.astype(BF16).astype(np.float32)
    wx_all = np.concatenate(
        [xb @ W_transform[k].astype(BF16).astype(np.float32)
         for k in range(2)], axis=1)
    wxT = wx_all.T
    ar_all = np.zeros((16, N_NODES), np.float32)
    al_all = np.zeros((16, N_NODES), np.float32)
    for k in range(2):
        ar_all[4 * k:4 * k + 3] = (wx_all[:, 64 * k:64 * k + 64] @
                                   w_right[k].T).T
        al_all[4 * k:4 * k + 3] = (wx_all[:, 64 * k:64 * k + 64] @
                                   w_left[k].T).T
    al_all[3] = 1.0
    al_all[7] = 1.0

    def segscan(parts, mrow):
        cs = np.cumsum(parts, axis=-1)
        starts = np.flatnonzero(mrow == 0.0)
        seg = np.cumsum(mrow == 0.0) - 1
        offs = np.take(cs[..., starts] - parts[..., starts], seg, axis=-1)
        return cs - offs

    outs = []
    for m in range(NC):
        im = in_maps[m]
        al_loc = al_all[:, m * NLOC:(m + 1) * NLOC]
        msum = np.zeros((128, NLOC), np.float64)
        dsum = np.zeros((2, NLOC), np.float64)
        for pc in range(NPH):
            rloc = im[f"rloc{pc}"].astype(np.int64)
            cloc = im[f"cloc{pc}"].astype(np.int64)
            alo = np.zeros((128, W), np.float32)
            aro = np.zeros((128, W), np.float32)
            for g in range(8):
                idx = rloc[16 * g:16 * g + 16].T.reshape(-1)
                alo[16 * g:16 * g + 16] = al_loc[:, idx]
                idxc = cloc[16 * g:16 * g + 16].T.reshape(-1)
                aro[16 * g:16 * g + 16] = ar_all[:, pc * CHUNK + idxc]
            alo = alo.astype(BF16).astype(np.float32)
            aro = aro.astype(BF16).astype(np.float32)
            p8 = (alo + aro) * im[f"vst{pc}"].astype(np.float32)
            s = p8[0::4] + p8[1::4] + p8[2::4] + p8[3::4]
            ex8 = np.exp(s)                       # [32, W]
            emask = im[f"_emask8{pc}"]
            exs = np.zeros((32, W), np.float32)
            for g in range(8):
                for k in (0, 1):
                    for h in (0, 1):
                        sl = slice(h * SEG, (h + 1) * SEG)
                        exs[4 * g + k, sl] = segscan(ex8[4 * g + k, sl],
                                                     emask[g, sl])
            dendw = im[f"dendw{pc}"].astype(np.int64)
            for g in range(8):
                idx = dendw[16 * g:16 * g + 16].T.reshape(-1)
                dsum[0] += exs[4 * g + 0, idx][:NLOC]
                dsum[1] += exs[4 * g + 1, idx][:NLOC]
            colw = im[f"colw{pc}"].astype(np.int64)
            cols = np.empty(TPH, np.int64)
            for t in range(NBLK):
                blkw = colw[0:16, t * (W // 16):(t + 1) * (W // 16)]
                cols[t * W:(t + 1) * W] = blkw.T.reshape(-1)
            pmask = im[f"_pmask8{pc}"]
            scanq = np.zeros((128, PTOT), np.float32)
            for t in range(8):
                idx = pc * CHUNK + cols[t * W:(t + 1) * W]
                g = wxT[:, idx]
                g = g * np.where((np.arange(128) < 64)[:, None],
                                 ex8[4 * t + 0], ex8[4 * t + 1])
                g = g.astype(BF16).astype(np.float32)
                part = g.reshape(128, PW, W4).sum(2)
                scanq[:, t * PW:(t + 1) * PW] = segscan(part, pmask[t])
            endw = im[f"endqw{pc}"].astype(np.int64)
            eidx = endw[0:16].T.reshape(-1)
            msum += scanq[:, eidx[:NLOC]]
        dsum += 1e-30
        out01 = msum.copy()
        out01[0:64] /= dsum[0]
        out01[64:128] /= dsum[1]
        xs = xb[m * NLOC:(m + 1) * NLOC]
        pre = (xs @ W_residual[:IN_DIM] +
               (out01[0:64] + out01[64:128]).T @ W_residual[IN_DIM:])
        out = np.where(pre > 0, pre, np.exp(np.minimum(pre, 0)) - 1)
        outs.append(out.astype(np.float32))
    return np.concatenate(outs, axis=0)


# ======================================================================
# Bass kernel builder
# ======================================================================

def build_bass():
    import sys
    if '/opt/trn_rl_repo' not in sys.path:
        sys.path.insert(0, '/opt/trn_rl_repo')
    from concourse import bass, bacc, tile, mybir

    dt = mybir.dt
    AL = mybir.AluOpType
    AF = mybir.ActivationFunctionType

    nc = bacc.Bacc(None, target_bir_lowering=False)

    def din(name, shape, d):
        return nc.dram_tensor(name, list(shape), d, kind="ExternalInput")

    xT_d = din("xT", (128, NLOC), dt.bfloat16)
    W01_d = din("W01", (128, 128), dt.bfloat16)
    LALX_d = din("LALX", (128, 128), dt.bfloat16)
    LARX_d = din("LARX", (128, 16), dt.bfloat16)
    WRT_d = din("WRT", (128, 64), dt.bfloat16)
    WRB_d = din("WRB", (128, 64), dt.bfloat16)
    CSEL_d = din("CSEL", (128, 128), dt.bfloat16)
    DSEL2_d = din("DSEL2", (128, 128), dt.bfloat16)
    W2_d = din("W2", (2, 128), dt.bfloat16)
    ONESROW_d = din("ONESROW", (1, NLOC), dt.float32)
    ph_d = []
    for pc in range(NPH):
        dd = dict(
            colw=din(f"colw{pc}", (128, TPH // 16), dt.int16),
            rloc=din(f"rloc{pc}", (128, W // 16), dt.int16),
            cloc=din(f"cloc{pc}", (128, W // 16), dt.int16),
            vst=din(f"vst{pc}", (128, W), dt.bfloat16),
            pmrep=din(f"pmrep{pc}", (128, PTOT), dt.bfloat16),
            emrep=din(f"emrep{pc}", (128, W), dt.bfloat16),
        )
        dd["endqw"] = din(f"endqw{pc}", (128, NLE // 16), dt.int16)
        dd["dendw"] = din(f"dendw{pc}", (128, NLE // 16), dt.int16)
        ph_d.append(dd)
    out_d = nc.dram_tensor("out", [64, NLOC], dt.float32,
                           kind="ExternalOutput")
    aginw = nc.dram_tensor("aginw", [NLOC, 128], dt.bfloat16)
    agoutw = nc.dram_tensor("agoutw", [N_NODES, 128], dt.bfloat16,
                            addr_space="Shared")
    aginr = nc.dram_tensor("aginr", [16, NLOC], dt.float32)
    agoutr = nc.dram_tensor("agoutr", [16 * NC, NLOC], dt.float32,
                            addr_space="Shared")

    NT512 = (NLOC + 511) // 512
    NJ = (NLE + 511) // 512

    with tile.TileContext(nc) as tc:
      with nc.allow_low_precision(reason="bf16 accums validated in emulation"):
        with (
            tc.tile_pool(name="big", bufs=1) as big,
            tc.tile_pool(name="alr", bufs=1) as alr,
            tc.tile_pool(name="res", bufs=1) as res,
            tc.tile_pool(name="mid", bufs=1) as mid,
            tc.tile_pool(name="work", bufs=2) as work,
            tc.tile_pool(name="one", bufs=1) as one,
            tc.tile_pool(name="psum", bufs=2, space="PSUM") as psum,
        ):
            # ---------- stage 1: wx, al, ar, collective ----------
            altab = alr.tile([128, NLOC], dt.float32, tag="altab")
            W01 = mid.tile([128, 128], dt.bfloat16, tag="w128")
            LALXt = mid.tile([128, 128], dt.bfloat16, tag="w128b")
            LARXt = mid.tile([128, 16], dt.bfloat16, tag="w16")
            nc.sync.dma_start(W01[:], W01_d[:])
            nc.sync.dma_start(LALXt[:], LALX_d[:])
            nc.sync.dma_start(LARXt[:], LARX_d[:])
            for j in range(NT512):
                a, b = j * 512, min(NLOC, (j + 1) * 512)
                xch = one.tile([128, 512], dt.bfloat16, tag="vmask")
                nc.sync.dma_start(xch[:, :b - a], xT_d[:, a:b])
                pw = psum.tile([128, 512], dt.float32, tag="pw")
                nc.tensor.matmul(pw[:, :b - a], W01[:], xch[:, :b - a],
                                 start=True, stop=True)
                wxb = one.tile([128, 512], dt.bfloat16, tag="p8x")
                nc.vector.tensor_copy(wxb[:, :b - a], pw[:, :b - a])
                for j4 in range(4):
                    na = a + j4 * 128
                    nb = min(b, na + 128)
                    if nb <= na:
                        continue
                    ntile = work.tile([128, 128], dt.bfloat16, tag="ntile")
                    nc.sync.dma_start(ntile[:],
                                      wxb[:, j4 * 128:(j4 + 1) * 128],
                                      transpose=True)
                    nc.scalar.dma_start(aginw[na:nb, :], ntile[:nb - na, :])
                pa = psum.tile([128, 512], dt.float32, tag="pw")
                nc.tensor.matmul(pa[:, :b - a], LALXt[:], xch[:, :b - a],
                                 start=True, stop=True)
                nc.scalar.activation(altab[:, a:b], pa[:, :b - a], AF.Copy)
                pr = psum.tile([16, 512], dt.float32, tag="pw")
                nc.tensor.matmul(pr[:, :b - a], LARXt[:], xch[:, :b - a],
                                 start=True, stop=True)
                ar16s = one.tile([16, 512], dt.float32, tag="ppseg")
                nc.scalar.activation(ar16s[:, :b - a], pr[:, :b - a], AF.Copy)
                nc.sync.dma_start(aginr[:, a:b], ar16s[:, :b - a])
            for g8 in range(8):
                nc.sync.dma_start(altab[16 * g8 + 3:16 * g8 + 4, :],
                                  ONESROW_d[:])
                nc.sync.dma_start(altab[16 * g8 + 7:16 * g8 + 8, :],
                                  ONESROW_d[:])
            nc.gpsimd.collective_compute(
                "AllGather", AL.bypass,
                replica_groups=[list(range(NC))],
                ins=[aginw.ap().opt()],
                outs=[agoutw.ap().opt()],
            )
            nc.gpsimd.collective_compute(
                "AllGather", AL.bypass,
                replica_groups=[list(range(NC))],
                ins=[aginr.ap().opt()],
                outs=[agoutr.ap().opt()],
            )

            msum = res.tile([128, NLOC], dt.bfloat16, tag="msum")
            dacc = res.tile([128, NLE], dt.bfloat16, tag="dacc")
            nc.vector.memset(msum[:], 0.0)
            nc.vector.memset(dacc[:], 0.0)
            CSELt = mid.tile([128, 128], dt.bfloat16, tag="csel")
            nc.sync.dma_start(CSELt[:], CSEL_d[:])
            W2t = mid.tile([2, 128], dt.bfloat16, tag="w2")
            nc.sync.dma_start(W2t[:], W2_d[:])

            for pc in range(NPH):
                pd = ph_d[pc]
                rloc = mid.tile([128, W // 16], dt.int16, tag="rloc")
                cloc = mid.tile([128, W // 16], dt.int16, tag="cloc")
                nc.sync.dma_start(rloc[:], pd["rloc"][:])
                nc.sync.dma_start(cloc[:], pd["cloc"][:])
                # --- B: score-land on the (time-shared) big slot: artab ---
                artab = big.tile([128, CHUNK], dt.float32, tag="big")
                for jj in range(2):
                    rk = 2 * pc + jj
                    for g in range(8):
                        nc.scalar.dma_start(
                            artab[16 * g:16 * g + 16,
                                  jj * NLOC:(jj + 1) * NLOC],
                            agoutr[rk * 16:rk * 16 + 16, :])
                ex8 = res.tile([128, W], dt.bfloat16, tag="ex8")
                alb16 = one.tile([128, W], dt.bfloat16, tag="alo")
                for s in range(W // SUBA):
                    a, b = s * SUBA, (s + 1) * SUBA
                    sw = SUBA // 16
                    albuf = one.tile([128, 2560], dt.float32, tag="gath")
                    nc.gpsimd.ap_gather(albuf[:, 0:SUBA], altab[:],
                                        rloc[:, s * sw:(s + 1) * sw],
                                        channels=128, num_elems=NLOC, d=1,
                                        num_idxs=SUBA)
                    nc.vector.tensor_copy(alb16[:, a:b], albuf[:, 0:SUBA])
                for s in range(W // SUBA):
                    a, b = s * SUBA, (s + 1) * SUBA
                    sw = SUBA // 16
                    aro = one.tile([128, 2560], dt.float32, tag="gath")
                    nc.gpsimd.ap_gather(aro[:, 0:SUBA], artab[:],
                                        cloc[:, s * sw:(s + 1) * sw],
                                        channels=128, num_elems=CHUNK, d=1,
                                        num_idxs=SUBA)
                    p8 = one.tile([128, SUBA], dt.bfloat16, tag="p8x")
                    nc.vector.tensor_tensor(p8[:], aro[:, 0:SUBA],
                                            alb16[:, a:b], AL.add)
                    vsts = one.tile([128, 2560], dt.bfloat16, tag="vmask")
                    nc.sync.dma_start(vsts[:, 0:SUBA], pd["vst"][:, a:b])
                    nc.vector.tensor_tensor(p8[:], p8[:],
                                            vsts[:, 0:SUBA], AL.mult)
                    for va, vb in ((0, 512), (512, 1024), (1024, 1152)):
                        sxp = psum.tile([128, 512], dt.float32, tag="pw")
                        nc.tensor.matmul(sxp[:, :vb - va], CSELt[:],
                                         p8[:, va:vb],
                                         start=True, stop=True)
                        nc.scalar.activation(ex8[:, a + va:a + vb],
                                             sxp[:, :vb - va], AF.Exp)
                # wxtab load can overlap the rest of B (WAR on big slot)
                exs = res.tile([128, W], dt.float32, tag="exsf")
                for h in (0, 1):
                    emaskh = one.tile([128, 2560], dt.bfloat16, tag="vmask")
                    nc.sync.dma_start(emaskh[:, 0:SEG],
                                      pd["emrep"][:, h * SEG:(h + 1) * SEG])
                    hs = slice(h * SEG, (h + 1) * SEG)
                    nc.vector.tensor_tensor_scan(
                        exs[:, hs], emaskh[:, 0:SEG], ex8[:, hs], 0.0,
                        op0=AL.mult, op1=AL.add)
                # --- C: main gather (transposed dma_gather) + msgs ---

                scanq = res.tile([128, PTOT], dt.float32, tag="sh16")
                for q in range(NWIN):
                    for tt in range(2):
                        t = 2 * q + tt
                        for sg in range(2):      # segs (= SUBM) within block
                            e0 = t * W + sg * SEG
                            w0 = e0 // 16
                            gt = one.tile([128, 9, 1, 256], dt.bfloat16,
                                          tag="gtt")
                            colws = work.tile([128, SUBM // 16], dt.int16,
                                              tag="colws")
                            nc.sync.dma_start(colws[:],
                                              pd["colw"][:, w0:w0 + SUBM // 16])
                            for dg in range(9):
                                nc.gpsimd.dma_gather(
                                    gt[:, dg, :, :],
                                    agoutw[pc * CHUNK:(pc + 1) * CHUNK, :],
                                    colws[:, dg * 16:(dg + 1) * 16],
                                    num_idxs=256, num_idxs_reg=256,
                                    elem_size=128, transpose=True)
                            gt = gt[:].rearrange("p a b c -> p (a b c)")
                            c0 = sg * SEG
                            ppseg = one.tile([128, SUBM // 4], dt.float32,
                                             tag="ppseg")
                            for v2 in range(2):
                                va = v2 * 1152
                                exfm = one.tile([2, 1152], dt.bfloat16,
                                                tag="p8x")
                                nc.sync.dma_start(
                                    exfm[0:1, :],
                                    ex8[16 * t:16 * t + 1,
                                        c0 + va:c0 + va + 1152])
                                nc.sync.dma_start(
                                    exfm[1:2, :],
                                    ex8[16 * t + 4:16 * t + 5,
                                        c0 + va:c0 + va + 1152])
                                exrep = psum.tile([128, 1152], dt.float32,
                                                  tag="exrep")
                                for v3 in range(3):
                                    sa = v3 * 512
                                    sb = min(1152, sa + 512)
                                    nc.tensor.matmul(
                                        exrep[:, sa:sb], W2t[:],
                                        exfm[:, sa:sb],
                                        start=True, stop=True)
                                gb = one.tile([128, 288, 4],
                                              dt.bfloat16, tag="gb")
                                g2o = gb[:].rearrange("p a b -> p (a b)")
                                nc.vector.tensor_tensor(
                                    g2o[:, :], gt[:, va:va + 1152],
                                    exrep[:], AL.mult)
                                nc.vector.tensor_reduce(
                                    ppseg[:, v2 * 288:(v2 + 1) * 288],
                                    gb[:], axis=mybir.AxisListType.X,
                                    op=AL.add)
                            pb = t * PW + sg * (PW // 2)
                            pglob = t * PW + sg * (PW // 2)
                            mkrs = one.tile([128, 2560], dt.bfloat16,
                                            tag="vmask")
                            nc.sync.dma_start(
                                mkrs[:, 0:PW // 2],
                                pd["pmrep"][:, pglob:pglob + PW // 2])
                            nc.vector.tensor_tensor_scan(
                                scanq[:, pb:pb + PW // 2],
                                mkrs[:, 0:PW // 2], ppseg[:],
                                0.0, op0=AL.mult, op1=AL.add)
                dendw = mid.tile([128, NLE // 16], dt.int16, tag="endidx")
                nc.sync.dma_start(dendw[:], pd["dendw"][:])
                for ea, en in ((0, 3136), (3136, 3120)):
                    endout = one.tile([128, 3136], dt.float32, tag="alo")
                    nc.gpsimd.ap_gather(
                        endout[:, 0:en], exs[:],
                        dendw[:, ea // 16:(ea + en) // 16],
                        channels=128, num_elems=W, d=1, num_idxs=en)
                    nc.vector.tensor_tensor(dacc[:, ea:ea + en],
                                            dacc[:, ea:ea + en],
                                            endout[:, 0:en], AL.add)
                endw = mid.tile([128, NLE // 16], dt.int16, tag="endidx")
                nc.sync.dma_start(endw[:], pd["endqw"][:])
                for ea, en in ((0, 3136), (3136, 3120)):
                    bb = min(NLOC, ea + en)
                    endout2 = one.tile([128, 3136], dt.float32, tag="alo")
                    nc.gpsimd.ap_gather(
                        endout2[:, 0:en], scanq[:],
                        endw[:, ea // 16:(ea + en) // 16],
                        channels=128, num_elems=PTOT, d=1, num_idxs=en)
                    nc.vector.tensor_tensor(
                        msum[:, ea:bb], msum[:, ea:bb],
                        endout2[:, 0:bb - ea], AL.add)

            # ---------- stage 4: divide + residual + elu ----------
            DSEL2t = mid.tile([128, 128], dt.bfloat16, tag="dsel2")
            nc.sync.dma_start(DSEL2t[:], DSEL2_d[:])
            WRTt = mid.tile([128, 64], dt.bfloat16, tag="w128")
            WRBt = mid.tile([128, 64], dt.bfloat16, tag="w128b")
            nc.sync.dma_start(WRTt[:], WRT_d[:])
            nc.sync.dma_start(WRBt[:], WRB_d[:])
            for j in range(NT512):
                a, b = j * 512, min(NLOC, (j + 1) * 512)
                xch = one.tile([128, 512], dt.bfloat16, tag="vmask")
                nc.sync.dma_start(xch[:, :b - a], xT_d[:, a:b])
                drp = psum.tile([128, 512], dt.float32, tag="pw")
                nc.tensor.matmul(drp[:, :b - a], DSEL2t[:], dacc[:, a:b],
                                 start=True, stop=True)
                rec = one.tile([128, PW // 2], dt.float32, tag="ppseg")
                nc.vector.tensor_scalar(rec[:, :b - a], drp[:, :b - a],
                                        1e-8, None, AL.add)
                nc.vector.reciprocal(rec[:, :b - a], rec[:, :b - a])
                msb = one.tile([128, 512], dt.bfloat16, tag="gb")
                nc.vector.tensor_tensor(msb[:, :b - a], msum[:, a:b],
                                        rec[:, :b - a], AL.mult)
                prr = psum.tile([64, 512], dt.float32, tag="pw")
                nc.tensor.matmul(prr[:, :b - a], WRTt[:], xch[:, :b - a],
                                 start=True, stop=False)
                nc.tensor.matmul(prr[:, :b - a], WRBt[:], msb[:, :b - a],
                                 start=False, stop=True)
                et = one.tile([128, PW // 2], dt.float32, tag="ppseg")
                nc.scalar.activation(et[0:64, :b - a], prr[:, :b - a], AF.Exp)
                nc.vector.tensor_scalar(et[0:64, :b - a], et[0:64, :b - a],
                                        -1.0, 0.0, AL.add, AL.min)
                nc.vector.tensor_scalar(prr[:, :b - a], prr[:, :b - a],
                                        0.0, None, AL.max)
                osb = one.tile([64, 512], dt.float32, tag="gb")
                nc.vector.tensor_tensor(osb[:, :b - a], et[0:64, :b - a],
                                        prr[:, :b - a], AL.add)
                nc.sync.dma_start(out_d[:, a:b], osb[:, :b - a])

    nc.compile()
    return nc


_CACHED = {}


def kernel(**inputs):
    import sys
    if '/opt/trn_rl_repo' not in sys.path:
        sys.path.insert(0, '/opt/trn_rl_repo')
    from concourse import bass_utils

    np_inputs = {k: np.asarray(v) for k, v in inputs.items()}
    in_maps = host_prep(**np_inputs)
    in_maps = [{k: v for k, v in im.items() if not k.startswith("_")}
               for im in in_maps]
    if 'nc' not in _CACHED:
        _CACHED['nc'] = build_bass()
    nc = _CACHED['nc']
    res = bass_utils.run_bass_kernel_spmd(nc, in_maps,
                                          core_ids=list(range(NC)))
    outs = [res.results[m]["out"] for m in range(NC)]
    return np.concatenate([o.T for o in outs], axis=0).astype(np.float32)
